# revision 82
# baseline (speedup 1.0000x reference)
"""CalderaLinear fused kernel for 8 Trainium2 NeuronCores — fp8 DoubleRow.

Math (reference): y = x @ Q^T + (x @ R^T) @ L^T + bias, with Q/L/R groupwise
int-dequantized (codes 0..15, group size 128).

Strategy (v2, fp8):
  * Column-parallel over d_out: core c owns out-features [c*512, (c+1)*512).
  * W_c = Q_c^T + R^T L_c^T ([d_in, 512]) has column means ~3600 (the R^T L^T
    product of non-negative codes) while the fluctuation around the mean is
    only ~270 rms.  fp8 e4m3's 3-bit mantissa on raw W gives ~2.5e-2 rel
    error (fails the 2e-2 gate), but on the *centered* W it gives ~4e-3.
    So the kernel computes, all on device:
        m_r   = mean_i r_deq[r, i]                    (DVE reduce)
        What  = Q^T/16 + (R - 1 m)^T (L/16)^T          (PE + DVE, cast e4m3)
        mu_o  = 16 * m @ (L/16)^T                      (PE, rank-1 weights)
        S_t   = sum_i x[t, i]                          (DVE reduce over bf16 x)
        y     = (16x)_fp8 @ What_fp8 + S * mu + bias   (PE DoubleRow + DVE/ACT)
    The rank-1 S*mu term restores the removed mean exactly; the fp8 rounding
    only ever touches the small fluctuating part.  Simulated rel_l2 ~4e-3.
  * The main matmul runs in MatmulPerfMode.DoubleRow: both operands e4m3,
    3D APs [128, 2, free] carrying two contraction planes per partition,
    contraction 256 per matmul -> half the matmul count of bf16.
  * S is data-parallel: core c reduces its own 1024 token rows (raw-layout
    bf16 x shard), then a 4 KB AllGather shares all 8192 sums.  Evictions
    are split: psum+bias -> SBUF immediately (frees PSUM), the +S*mu
    correction + store lag one slab so the collective latency hides.
  * x streams as pre-tiled fp8 [128, 2, 512] blocks; W stays SBUF-resident
    fp8; PSUM accumulates over the 16 doubled k-tiles; bias fused into
    eviction.  Host side only reshapes/transposes/casts/scales-by-2^±4 and
    concatenates the 8 output shards: all dequant + matmul + reduction math
    runs on the NeuronCores.
"""

import numpy as np
import ml_dtypes

P = 128
D_IN = 4096
D_OUT = 4096
TOK = 8192
RANK = 256
NCORES = 8
OC = D_OUT // NCORES      # 512 out features per core
KT = D_IN // P            # 32 contraction tiles (bf16 build granularity)
KKT = KT // 2             # 16 doubled contraction tiles (fp8 DoubleRow)
MS = 512                  # token slab
NS = TOK // MS            # 16 slabs
SUB = MS // P             # 4 psum sub-tiles per slab
RG = RANK // 128          # 2 rank tiles
RCH = 8                   # R chunks along d_in
RCW = D_IN // RCH         # 512 R columns per chunk
SHTOK = TOK // NCORES     # 1024 tokens per core for the S reduction
SHT = SHTOK // P          # 8 row-tiles of the S shard

_module_cache = {}
last_result = None


def _build_module():
    import concourse.mybir as mybir
    import concourse.tile as tile
    from concourse import bacc

    bf = mybir.dt.bfloat16
    f8 = mybir.dt.float8e4
    f32 = mybir.dt.float32
    AX = mybir.AxisListType
    DR = mybir.MatmulPerfMode.DoubleRow
    COPY = mybir.ActivationFunctionType.Copy

    nc = bacc.Bacc(None, target_bir_lowering=False, debug=False,
                   num_devices=NCORES)
    xt_d = nc.dram_tensor("xt", (NS, KKT, P, 2, MS), f8, kind="ExternalInput")
    lcod_d = nc.dram_tensor("lcod", (P, RG, OC), f8, kind="ExternalInput")
    lsc_d = nc.dram_tensor("lsc", (1, RG, OC), bf, kind="ExternalInput")
    wbr_d = nc.dram_tensor("wbr", (P, RCH, RG, 2, RCW), f8, kind="ExternalInput")
    wbq_d = nc.dram_tensor("wbq", (P, KT, 2, OC), f8, kind="ExternalInput")
    xb_d = nc.dram_tensor("xb", (SHT // 2, P, 2, D_IN), bf,
                          kind="ExternalInput")
    biasr_d = nc.dram_tensor("biasr", (1, OC), bf, kind="ExternalInput")
    y_d = nc.dram_tensor("y", (TOK, OC), f32, kind="ExternalOutput")

    with tile.TileContext(nc) as tc:
        with (
            tc.tile_pool(name="const", bufs=1) as const,
            tc.tile_pool(name="wpool", bufs=1) as wpool,
            tc.tile_pool(name="xpool", bufs=16) as xpool,
            tc.tile_pool(name="xbpool", bufs=3) as xbpool,
            tc.tile_pool(name="qpool", bufs=4) as qpool,
            tc.tile_pool(name="ypool", bufs=32) as ypool,
            tc.tile_pool(name="cpool", bufs=4) as cpool,
            tc.tile_pool(name="ppool", bufs=6, space="PSUM") as ppool,
            tc.tile_pool(name="wbpool", bufs=2, space="PSUM") as wbpool,
            tc.tile_pool(name="dpool", bufs=1, space="DRAM") as dpool,
        ):
            sin_d = dpool.tile([P, SHT], f32, name="sin")
            sout_d = dpool.tile([NCORES, P, SHT], f32, name="sout",
                                addr_space="Shared")
            Lcod = const.tile([P, RG, OC], f8)
            Lscr = const.tile([1, RG, OC], bf)
            biasr = const.tile([1, OC], bf)
            WBR = const.tile([P, RCH, RG, 2, RCW], f8)
            RD = const.tile([P, RCH, RG, RCW], bf)
            bias_t = const.tile([P, OC], f32)
            LdT = const.tile([P, RG, OC], bf)
            Wt = wpool.tile([P, KKT, 2, OC], f8)
            mneg = const.tile([P, RG], f32)     # -mean est. (chunk 0 cols)
            m16 = const.tile([P, RG], bf)       # +m estimate (bf16, mu lhsT)
            wu = const.tile([P, OC], bf)        # warmup scratch
            ones1 = const.tile([1, P], bf)
            murow = const.tile([1, OC], bf)
            munegrow = const.tile([1, OC], bf)
            mub = const.tile([P, OC], f32)      # broadcast 16*mu/16 = mu
            S_sb = const.tile([P, NS * SUB], f32)
            Sown = const.tile([P, SHT], f32)

            # ---- phase-0 DMAs.  sync: R chunk 0 + L header + bias row
            # (small, feeds the critical m~/W-build chain); scalar: chunk 1;
            # gpsimd: R chunks 2-7, then the bf16 x shard for S.
            nc.sync.dma_start(WBR[:, 0], wbr_d[:, 0])
            nc.scalar.dma_start(WBR[:, 1], wbr_d[:, 1])
            nc.sync.dma_start(Lcod[:], lcod_d[:])
            nc.sync.dma_start(Lscr[:], lsc_d[:])
            nc.sync.dma_start(biasr[:], biasr_d[:])
            for ch in range(2, RCH):
                nc.gpsimd.dma_start(WBR[:, ch], wbr_d[:, ch])
            xbt = []
            for j in range(SHT // 2):
                t = xbpool.tile([P, 2, D_IN], bf, tag="xb", name=f"xb{j}")
                nc.gpsimd.dma_start(t[:], xb_d[j])
                xbt.append(t)

            # ---- PE warmup: HAM un-throttles after ~3.4us of activity, so
            # burn idle DMA-wait time on dummy matmuls at the very start.
            nc.vector.memset(wu[:], 0.0)
            wu_ps = wbpool.tile([P, OC], f32, tag="wb", name="wups")
            for i in range(12):
                nc.tensor.matmul(wu_ps[:], wu[:, 0:P], wu[:],
                                 start=True, stop=True)

            nc.vector.memset(ones1[:], 1.0)

            # ---- L^T dequant: broadcast the [1, OC] scale rows to all 128
            # partitions with a K=1 ones-matmul, then codes x scales on DVE
            sc_ps = []
            for j in range(RG):
                ps = wbpool.tile([P, OC], f32, tag="wb", name=f"lsc{j}")
                nc.tensor.matmul(ps[:], ones1[:], Lscr[:, j, :],
                                 start=True, stop=True)
                sc_ps.append(ps)
            for j in range(RG):
                nc.vector.tensor_mul(LdT[:, j, :], Lcod[:, j, :],
                                     sc_ps[j][:])

            # ---- mean estimate from R chunk 0 only.  The centering identity
            # x@(W - 1 m~ L16) + S*(m~ L16) == x@W holds exactly for ANY m~;
            # a 512-column estimate only leaves a negligible rank-1 residual
            # in the fp8 rounding, and it kills the full-R DMA dependency.
            def deq_center(ch):
                for j in range(RG):
                    nc.vector.tensor_mul(RD[:, ch, j, :],
                                         WBR[:, ch, j, 0, :],
                                         WBR[:, ch, j, 1, :])
                    nc.vector.tensor_scalar_add(RD[:, ch, j, :],
                                                RD[:, ch, j, :],
                                                mneg[:, j:j + 1])

            for j in range(RG):
                nc.vector.tensor_mul(RD[:, 0, j, :], WBR[:, 0, j, 0, :],
                                     WBR[:, 0, j, 1, :])
                nc.vector.reduce_sum(mneg[:, j:j + 1], RD[:, 0, j, :],
                                     axis=AX.X, negate=True)
            nc.vector.tensor_scalar_mul(m16[:], mneg[:], -1.0 / RCW)
            nc.vector.tensor_scalar_mul(mneg[:], mneg[:], 1.0 / RCW)
            for j in range(RG):
                nc.vector.tensor_scalar_add(RD[:, 0, j, :], RD[:, 0, j, :],
                                            mneg[:, j:j + 1])

            # ---- mu = 16 * m @ (L/16)^T, broadcast to all 128 partitions
            mu_ps = wbpool.tile([P, OC], f32, tag="wb", name="mu1")
            for j in range(RG):
                nc.tensor.matmul(mu_ps[0:1, :], m16[:, j:j + 1], LdT[:, j, :],
                                 start=(j == 0), stop=(j == RG - 1))
            nc.vector.tensor_copy(murow[:], mu_ps[0:1, :])
            nc.vector.tensor_scalar_mul(munegrow[:], mu_ps[0:1, :], -1.0)
            mub_ps = wbpool.tile([P, OC], f32, tag="wb", name="mu2")
            nc.tensor.matmul(mub_ps[:], ones1[:], murow[:], start=True,
                             stop=True)
            nc.vector.tensor_scalar_mul(mub[:], mub_ps[:], 16.0)

            # ---- bias broadcast [1, OC] -> [128, OC] the same way
            bb_ps = wbpool.tile([P, OC], f32, tag="wb", name="bb")
            nc.tensor.matmul(bb_ps[:], ones1[:], biasr[:], start=True,
                             stop=True)
            nc.vector.tensor_copy(bias_t[:], bb_ps[:])

            # ---- W build: What k-tile k = R_hat^T(L/16)^T + Q^T/16 -> e4m3
            def rd(j, k):
                return RD[:, k // 4, j, (k % 4) * P:(k % 4) * P + P]

            def build_w(k):
                ps = wbpool.tile([P, OC], f32, tag="wb", name=f"wb{k}")
                for j in range(RG):
                    nc.tensor.matmul(ps[:], rd(j, k), LdT[:, j, :],
                                     start=(j == 0), stop=(j == RG - 1))
                qt = qpool.tile([P, 2, OC], f8, tag="qt")
                eng = nc.sync if k % 2 == 0 else nc.scalar
                eng.dma_start(qt[:], wbq_d[:, k])
                nc.vector.tensor_mul(qt[:, 0, :], qt[:, 0, :], qt[:, 1, :])
                nc.vector.tensor_add(Wt[:, k // 2, k % 2, :], ps[:],
                                     qt[:, 0, :])

            psums = {}

            def slab_mms(s, dma_engines):
                psums[s] = [ppool.tile([P, OC], f32, tag="ps",
                                       name=f"ps{s}_{i}") for i in range(SUB)]
                for kk in range(KKT):
                    xt = xpool.tile([P, 2, MS], f8, tag="x")
                    dma_engines[kk % len(dma_engines)].dma_start(
                        xt[:], xt_d[s, kk])
                    for sub in range(SUB):
                        nc.tensor.matmul(
                            psums[s][sub][:],
                            xt[:, :, sub * P:(sub + 1) * P],
                            Wt[:, kk, :, :],
                            start=(kk == 0), stop=(kk == KKT - 1),
                            perf_mode=DR,
                        )
                    yield kk

            ytiles = {}

            def evict_a(s):
                # psum + bias -> SBUF, frees the psum bank
                ytiles[s] = []
                for sub in range(SUB):
                    yt = ypool.tile([P, OC], f32, tag="y", name=f"y{s}_{sub}")
                    nc.vector.tensor_add(yt[:], psums[s][sub][:], bias_t[:])
                    ytiles[s].append(yt)

            def evict_b(s):
                # + S*mu (ACT outer product), then store
                for sub in range(SUB):
                    col = s * SUB + sub
                    corr = cpool.tile([P, OC], f32, tag="c")
                    nc.scalar.activation(corr[:], mub[:], COPY,
                                         scale=S_sb[:, col:col + 1])
                    yt = ytiles[s][sub]
                    nc.vector.tensor_add(yt[:], yt[:], corr[:])
                    # y stores ride the gpsimd queue (idle after the x shard),
                    # keeping sync/scalar free for the x8 stream; the last two
                    # slabs fan out across all three queues to shrink the tail
                    eng = nc.gpsimd if s < NS - 3 else (
                        [nc.gpsimd, nc.scalar, nc.sync][sub % 3])
                    eng.dma_start(
                        y_d[s * MS + sub * P:s * MS + (sub + 1) * P, :], yt[:])

            # ---- S shard: reduce own 1024 bf16 token rows (spread across
            # slabs 1-4 so psum eviction never waits), AllGather 4 KB
            def s_reduce(j):
                nc.vector.reduce_sum(Sown[:, 2 * j:2 * j + 2], xbt[j][:],
                                     axis=AX.X)

            def s_finish():
                nc.gpsimd.dma_start(sin_d[:], Sown[:])
                nc.gpsimd.collective_compute(
                    "AllGather",
                    mybir.AluOpType.bypass,
                    replica_groups=[list(range(NCORES))],
                    ins=[sin_d.opt()],
                    outs=[sout_d.opt()],
                )
                for c in range(NCORES):
                    nc.gpsimd.dma_start(S_sb[:, c * SHT:(c + 1) * SHT],
                                        sout_d[c])

            # ---- slab 0: W build interleaved two tiles ahead, R chunks
            # dequantized + centered just-in-time as their DMAs land
            deq_center(1)
            for k in range(6):
                build_w(k)
            for kk in slab_mms(0, [nc.sync, nc.scalar]):
                if kk % 2 == 0 and kk // 2 + 2 < RCH:
                    deq_center(kk // 2 + 2)
                if kk < KKT - 3:
                    build_w(2 * kk + 6)
                    build_w(2 * kk + 7)

            evict_a(0)

            # evict_b lags ~7 slabs mid-run (rides out the S collective
            # latency), catches up two-per-slab from slab 10.  The backlog is
            # drained at the TOP of each iteration so it overlaps that slab's
            # matmuls, leaving only slab 15's evict_b after the last matmul.
            # ypool ring safety: live ytile slabs stay <= 8 = 32/SUB, and the
            # freeing evict_b is always emitted before the reusing evict_a.
            done_b = 0
            for s in range(1, NS):
                want = s - 7 if s < 9 else 2 * (s - 9) + 2
                while done_b <= min(want, s - 1):
                    evict_b(done_b)
                    done_b += 1
                for kk in slab_mms(s, [nc.sync, nc.scalar]):
                    pass
                evict_a(s)
                if s <= 4:
                    s_reduce(s - 1)
                    if s == 4:
                        s_finish()
            while done_b < NS:
                evict_b(done_b)
                done_b += 1

    nc.compile()
    return nc


def kernel(x, q_values, q_scales, l_values, l_scales, r_values, r_scales, bias,
           _trace=False):
    from concourse.bass_utils import run_bass_kernel_spmd

    bf16 = ml_dtypes.bfloat16
    e4m3 = ml_dtypes.float8_e4m3

    if "m" not in _module_cache:
        _module_cache["m"] = _build_module()
    nc = _module_cache["m"]

    # host-side marshaling (layout + dtype + power-of-two scaling only)
    x = np.asarray(x, dtype=np.float32)
    q_values = np.asarray(q_values)
    q_scales = np.asarray(q_scales, np.float32)
    l_values = np.asarray(l_values)
    l_scales = np.asarray(l_scales, np.float32)
    r_values = np.asarray(r_values)
    r_scales = np.asarray(r_scales, np.float32)
    bias = np.asarray(bias, np.float32)

    # x*16 as e4m3, tiled [NS, KKT, P, 2, MS]: plane ko holds i = kk*256+ko*128+p
    xs = np.clip(x * 16.0, -240.0, 240.0)
    xt8 = np.ascontiguousarray(
        xs.reshape(NS, MS, KKT, 2, P).transpose(0, 2, 4, 3, 1)
    ).astype(e4m3)
    # bf16 raw-row x for the S reduction, per-core shard below
    xb_all = x.astype(bf16)

    rs_full = np.repeat(r_scales, D_IN // r_scales.shape[1], axis=1)
    rv_f = r_values.astype(np.float32)
    # wbr[p, ch, j, 0, :] = r codes, [.., 1, :] = broadcast r scales
    wbr = np.empty((P, RCH, RG, 2, RCW), np.float32)
    for ch in range(RCH):
        cs = slice(ch * RCW, (ch + 1) * RCW)
        for j in range(RG):
            wbr[:, ch, j, 0, :] = rv_f[j * P:(j + 1) * P, cs]
            wbr[:, ch, j, 1, :] = rs_full[j * P:(j + 1) * P, cs]
    wbr = wbr.astype(e4m3)

    in_maps = []
    for c in range(NCORES):
        sl = slice(c * OC, (c + 1) * OC)
        qt_c = q_values[sl].T.astype(np.float32)            # [D_IN, OC]
        qst_c = (q_scales[sl].T / 16.0).astype(np.float32)  # [KT, OC]
        ltv_c = l_values[sl].T.astype(np.float32)           # [RANK, OC]
        lst_c = (l_scales[sl].T / 16.0).astype(np.float32)  # [RG, OC]

        lcod = np.empty((P, RG, OC), np.float32)
        for j in range(RG):
            lcod[:, j, :] = ltv_c[j * P:(j + 1) * P, :]
        wbq = np.empty((P, KT, 2, OC), np.float32)
        for k in range(KT):
            wbq[:, k, 0, :] = qt_c[k * P:(k + 1) * P, :]
            wbq[:, k, 1, :] = np.broadcast_to(qst_c[k], (P, OC))

        in_maps.append({
            "xt": xt8,
            "lcod": lcod.astype(e4m3),
            "lsc": np.ascontiguousarray(lst_c.reshape(1, RG, OC)).astype(bf16),
            "wbr": wbr,
            "wbq": wbq.astype(e4m3),
            "xb": np.ascontiguousarray(
                xb_all[c * SHTOK:(c + 1) * SHTOK]
                .reshape(SHT // 2, 2, P, D_IN).transpose(0, 2, 1, 3)),
            "biasr": np.ascontiguousarray(bias[sl].reshape(1, OC)).astype(bf16),
        })

    res = run_bass_kernel_spmd(
        nc, in_maps, core_ids=list(range(NCORES)), trace=_trace
    )
    global last_result
    last_result = res
    return np.concatenate([r["y"] for r in res.results], axis=1)


# revision 83
# speedup vs baseline: 1.0449x; 1.0449x over previous
"""CalderaLinear fused kernel for 8 Trainium2 NeuronCores — fp8 DoubleRow.

Math (reference): y = x @ Q^T + (x @ R^T) @ L^T + bias, with Q/L/R groupwise
int-dequantized (codes 0..15, group size 128).

Strategy (v2, fp8):
  * Column-parallel over d_out: core c owns out-features [c*512, (c+1)*512).
  * W_c = Q_c^T + R^T L_c^T ([d_in, 512]) has column means ~3600 (the R^T L^T
    product of non-negative codes) while the fluctuation around the mean is
    only ~270 rms.  fp8 e4m3's 3-bit mantissa on raw W gives ~2.5e-2 rel
    error (fails the 2e-2 gate), but on the *centered* W it gives ~4e-3.
    So the kernel computes, all on device:
        m_r   = mean_i r_deq[r, i]                    (DVE reduce)
        What  = Q^T/16 + (R - 1 m)^T (L/16)^T          (PE + DVE, cast e4m3)
        mu_o  = 16 * m @ (L/16)^T                      (PE, rank-1 weights)
        S_t   = sum_i x[t, i]                          (DVE reduce over bf16 x)
        y     = (16x)_fp8 @ What_fp8 + S * mu + bias   (PE DoubleRow + DVE/ACT)
    The rank-1 S*mu term restores the removed mean exactly; the fp8 rounding
    only ever touches the small fluctuating part.  Simulated rel_l2 ~4e-3.
  * The main matmul runs in MatmulPerfMode.DoubleRow: both operands e4m3,
    3D APs [128, 2, free] carrying two contraction planes per partition,
    contraction 256 per matmul -> half the matmul count of bf16.
  * S is data-parallel: core c reduces its own 1024 token rows (raw-layout
    bf16 x shard), then a 4 KB AllGather shares all 8192 sums.  Evictions
    are split: psum+bias -> SBUF immediately (frees PSUM), the +S*mu
    correction + store lag one slab so the collective latency hides.
  * x streams as pre-tiled fp8 [128, 2, 512] blocks; W stays SBUF-resident
    fp8; PSUM accumulates over the 16 doubled k-tiles; bias fused into
    eviction.  Host side only reshapes/transposes/casts/scales-by-2^±4 and
    concatenates the 8 output shards: all dequant + matmul + reduction math
    runs on the NeuronCores.
"""

import numpy as np
import ml_dtypes

P = 128
D_IN = 4096
D_OUT = 4096
TOK = 8192
RANK = 256
NCORES = 8
OC = D_OUT // NCORES      # 512 out features per core
KT = D_IN // P            # 32 contraction tiles (bf16 build granularity)
KKT = KT // 2             # 16 doubled contraction tiles (fp8 DoubleRow)
MS = 512                  # token slab
NS = TOK // MS            # 16 slabs
SUB = MS // P             # 4 psum sub-tiles per slab
RG = RANK // 128          # 2 rank tiles
RCH = 8                   # R chunks along d_in
RCW = D_IN // RCH         # 512 R columns per chunk
SHTOK = TOK // NCORES     # 1024 tokens per core for the S reduction
SHT = SHTOK // P          # 8 row-tiles of the S shard

_module_cache = {}
last_result = None


def _build_module():
    import concourse.mybir as mybir
    import concourse.tile as tile
    from concourse import bacc

    bf = mybir.dt.bfloat16
    f8 = mybir.dt.float8e4
    f32 = mybir.dt.float32
    AX = mybir.AxisListType
    DR = mybir.MatmulPerfMode.DoubleRow
    COPY = mybir.ActivationFunctionType.Copy

    nc = bacc.Bacc(None, target_bir_lowering=False, debug=False,
                   num_devices=NCORES)
    xt_d = nc.dram_tensor("xt", (NS, KKT, P, 2, MS), f8, kind="ExternalInput")
    lcod_d = nc.dram_tensor("lcod", (P, RG, OC), f8, kind="ExternalInput")
    lsc_d = nc.dram_tensor("lsc", (1, RG, OC), bf, kind="ExternalInput")
    wbr_d = nc.dram_tensor("wbr", (P, RCH, RG, 2, RCW), f8, kind="ExternalInput")
    wbq_d = nc.dram_tensor("wbq", (P, KT, 2, OC), f8, kind="ExternalInput")
    XBH = D_IN // 2
    xb_d = nc.dram_tensor("xb", (SHT, P, 2, XBH), bf, kind="ExternalInput")
    biasr_d = nc.dram_tensor("biasr", (1, OC), bf, kind="ExternalInput")
    y_d = nc.dram_tensor("y", (TOK, OC), f32, kind="ExternalOutput")

    with tile.TileContext(nc) as tc:
        with (
            tc.tile_pool(name="const", bufs=1) as const,
            tc.tile_pool(name="wpool", bufs=1) as wpool,
            tc.tile_pool(name="xpool", bufs=16) as xpool,
            tc.tile_pool(name="xbpool", bufs=8) as xbpool,
            tc.tile_pool(name="qpool", bufs=4) as qpool,
            tc.tile_pool(name="ypool", bufs=28) as ypool,
            tc.tile_pool(name="cpool", bufs=4) as cpool,
            tc.tile_pool(name="ppool", bufs=6, space="PSUM") as ppool,
            tc.tile_pool(name="wbpool", bufs=2, space="PSUM") as wbpool,
            tc.tile_pool(name="dpool", bufs=1, space="DRAM") as dpool,
        ):
            sin_d = dpool.tile([P, SHT], f32, name="sin")
            sout_d = dpool.tile([NCORES, P, SHT], f32, name="sout",
                                addr_space="Shared")
            Lcod = const.tile([P, RG, OC], f8)
            Lscr = const.tile([1, RG, OC], bf)
            biasr = const.tile([1, OC], bf)
            WBR = const.tile([P, RCH, RG, 2, RCW], f8)
            RD = const.tile([P, RCH, RG, RCW], bf)
            bias_t = const.tile([P, OC], f32)
            LdT = const.tile([P, RG, OC], bf)
            Wt = wpool.tile([P, KKT, 2, OC], f8)
            mneg = const.tile([P, RG], f32)     # -mean est. (chunk 0 cols)
            m16 = const.tile([P, RG], bf)       # +m estimate (bf16, mu lhsT)
            wu = const.tile([P, OC], bf)        # warmup scratch
            ones1 = const.tile([1, P], bf)
            murow = const.tile([1, OC], bf)
            munegrow = const.tile([1, OC], bf)
            mub = const.tile([P, OC], f32)      # broadcast 16*mu/16 = mu
            S_sb = const.tile([P, NS * SUB], f32)
            Sown = const.tile([P, SHT], f32)
            Spart = const.tile([P, SHT, 2], f32)

            # ---- phase-0 DMAs.  sync: R chunk 0 + L header + bias row
            # (small, feeds the critical m~/W-build chain); scalar: chunk 1;
            # gpsimd: R chunks 2-7, then the bf16 x shard for S.
            nc.sync.dma_start(WBR[:, 0], wbr_d[:, 0])
            nc.scalar.dma_start(WBR[:, 1], wbr_d[:, 1])
            nc.sync.dma_start(Lcod[:], lcod_d[:])
            nc.sync.dma_start(Lscr[:], lsc_d[:])
            nc.sync.dma_start(biasr[:], biasr_d[:])
            for ch in range(2, RCH):
                nc.gpsimd.dma_start(WBR[:, ch], wbr_d[:, ch])
            xbt = []
            for j in range(SHT):
                for h in range(2):
                    t = xbpool.tile([P, XBH], bf, tag="xb", name=f"xb{j}_{h}")
                    nc.gpsimd.dma_start(t[:], xb_d[j, :, h])
                    xbt.append(t)

            # ---- PE warmup: HAM un-throttles after ~3.4us of activity, so
            # burn idle DMA-wait time on dummy matmuls at the very start.
            nc.vector.memset(wu[:], 0.0)
            wu_ps = wbpool.tile([P, OC], f32, tag="wb", name="wups")
            for i in range(12):
                nc.tensor.matmul(wu_ps[:], wu[:, 0:P], wu[:],
                                 start=True, stop=True)

            nc.vector.memset(ones1[:], 1.0)

            # ---- L^T dequant: broadcast the [1, OC] scale rows to all 128
            # partitions with a K=1 ones-matmul, then codes x scales on DVE
            sc_ps = []
            for j in range(RG):
                ps = wbpool.tile([P, OC], f32, tag="wb", name=f"lsc{j}")
                nc.tensor.matmul(ps[:], ones1[:], Lscr[:, j, :],
                                 start=True, stop=True)
                sc_ps.append(ps)
            for j in range(RG):
                nc.vector.tensor_mul(LdT[:, j, :], Lcod[:, j, :],
                                     sc_ps[j][:])

            # ---- mean estimate from R chunk 0 only.  The centering identity
            # x@(W - 1 m~ L16) + S*(m~ L16) == x@W holds exactly for ANY m~;
            # a 512-column estimate only leaves a negligible rank-1 residual
            # in the fp8 rounding, and it kills the full-R DMA dependency.
            def deq_center(ch):
                for j in range(RG):
                    nc.vector.tensor_mul(RD[:, ch, j, :],
                                         WBR[:, ch, j, 0, :],
                                         WBR[:, ch, j, 1, :])
                    nc.vector.tensor_scalar_add(RD[:, ch, j, :],
                                                RD[:, ch, j, :],
                                                mneg[:, j:j + 1])

            for j in range(RG):
                nc.vector.tensor_mul(RD[:, 0, j, :], WBR[:, 0, j, 0, :],
                                     WBR[:, 0, j, 1, :])
                nc.vector.reduce_sum(mneg[:, j:j + 1], RD[:, 0, j, :],
                                     axis=AX.X, negate=True)
            nc.vector.tensor_scalar_mul(m16[:], mneg[:], -1.0 / RCW)
            nc.vector.tensor_scalar_mul(mneg[:], mneg[:], 1.0 / RCW)
            for j in range(RG):
                nc.vector.tensor_scalar_add(RD[:, 0, j, :], RD[:, 0, j, :],
                                            mneg[:, j:j + 1])

            # ---- mu = 16 * m @ (L/16)^T, broadcast to all 128 partitions
            mu_ps = wbpool.tile([P, OC], f32, tag="wb", name="mu1")
            for j in range(RG):
                nc.tensor.matmul(mu_ps[0:1, :], m16[:, j:j + 1], LdT[:, j, :],
                                 start=(j == 0), stop=(j == RG - 1))
            nc.vector.tensor_copy(murow[:], mu_ps[0:1, :])
            nc.vector.tensor_scalar_mul(munegrow[:], mu_ps[0:1, :], -1.0)
            mub_ps = wbpool.tile([P, OC], f32, tag="wb", name="mu2")
            nc.tensor.matmul(mub_ps[:], ones1[:], murow[:], start=True,
                             stop=True)
            nc.vector.tensor_scalar_mul(mub[:], mub_ps[:], 16.0)

            # ---- bias broadcast [1, OC] -> [128, OC] the same way
            bb_ps = wbpool.tile([P, OC], f32, tag="wb", name="bb")
            nc.tensor.matmul(bb_ps[:], ones1[:], biasr[:], start=True,
                             stop=True)
            nc.vector.tensor_copy(bias_t[:], bb_ps[:])

            # ---- W build: What k-tile k = R_hat^T(L/16)^T + Q^T/16 -> e4m3
            def rd(j, k):
                return RD[:, k // 4, j, (k % 4) * P:(k % 4) * P + P]

            def build_w(k):
                ps = wbpool.tile([P, OC], f32, tag="wb", name=f"wb{k}")
                for j in range(RG):
                    nc.tensor.matmul(ps[:], rd(j, k), LdT[:, j, :],
                                     start=(j == 0), stop=(j == RG - 1))
                qt = qpool.tile([P, 2, OC], f8, tag="qt")
                eng = nc.sync if k % 2 == 0 else nc.scalar
                eng.dma_start(qt[:], wbq_d[:, k])
                nc.vector.tensor_mul(qt[:, 0, :], qt[:, 0, :], qt[:, 1, :])
                nc.vector.tensor_add(Wt[:, k // 2, k % 2, :], ps[:],
                                     qt[:, 0, :])

            psums = {}

            def slab_mms(s, dma_engines):
                psums[s] = [ppool.tile([P, OC], f32, tag="ps",
                                       name=f"ps{s}_{i}") for i in range(SUB)]
                for kk in range(KKT):
                    xt = xpool.tile([P, 2, MS], f8, tag="x")
                    dma_engines[kk % len(dma_engines)].dma_start(
                        xt[:], xt_d[s, kk])
                    for sub in range(SUB):
                        nc.tensor.matmul(
                            psums[s][sub][:],
                            xt[:, :, sub * P:(sub + 1) * P],
                            Wt[:, kk, :, :],
                            start=(kk == 0), stop=(kk == KKT - 1),
                            perf_mode=DR,
                        )
                    yield kk

            ytiles = {}

            def evict_a(s):
                # psum + bias -> SBUF, frees the psum bank
                ytiles[s] = []
                for sub in range(SUB):
                    yt = ypool.tile([P, OC], f32, tag="y", name=f"y{s}_{sub}")
                    nc.vector.tensor_add(yt[:], psums[s][sub][:], bias_t[:])
                    ytiles[s].append(yt)

            def evict_b(s):
                # + S*mu (ACT outer product), then store
                for sub in range(SUB):
                    col = s * SUB + sub
                    corr = cpool.tile([P, OC], f32, tag="c")
                    nc.scalar.activation(corr[:], mub[:], COPY,
                                         scale=S_sb[:, col:col + 1])
                    yt = ytiles[s][sub]
                    nc.vector.tensor_add(yt[:], yt[:], corr[:])
                    # y stores ride the gpsimd queue (idle after the x shard),
                    # keeping sync/scalar free for the x8 stream; the last two
                    # slabs fan out across all three queues to shrink the tail
                    eng = nc.gpsimd if s < NS - 1 else (
                        nc.scalar if sub % 2 == 0 else nc.sync)
                    eng.dma_start(
                        y_d[s * MS + sub * P:s * MS + (sub + 1) * P, :], yt[:])

            # ---- S shard: reduce own 1024 bf16 token rows (spread across
            # slabs 1-4 so psum eviction never waits), AllGather 4 KB
            def s_reduce(i):
                h = i % 2
                nc.vector.reduce_sum(Spart[:, i // 2, h:h + 1], xbt[i][:],
                                     axis=AX.X)

            def s_finish():
                nc.vector.reduce_sum(Sown[:], Spart[:], axis=AX.X)
                nc.gpsimd.dma_start(sin_d[:], Sown[:])
                nc.gpsimd.collective_compute(
                    "AllGather",
                    mybir.AluOpType.bypass,
                    replica_groups=[list(range(NCORES))],
                    ins=[sin_d.opt()],
                    outs=[sout_d.opt()],
                )
                for c in range(NCORES):
                    nc.gpsimd.dma_start(S_sb[:, c * SHT:(c + 1) * SHT],
                                        sout_d[c])

            # ---- slab 0: W build interleaved two tiles ahead, R chunks
            # dequantized + centered just-in-time as their DMAs land
            for k in range(4):
                build_w(k)
            for kk in slab_mms(0, [nc.sync, nc.scalar]):
                if kk % 2 == 0 and kk // 2 + 1 < RCH:
                    deq_center(kk // 2 + 1)
                if kk >= 12 and kk % 2 == 0:
                    s_reduce(kk // 2 - 6)
                if kk < KKT - 2:
                    build_w(2 * kk + 4)
                    build_w(2 * kk + 5)

            evict_a(0)

            # evict_b lags ~7 slabs mid-run (rides out the S collective
            # latency), catches up two-per-slab from slab 10.  The backlog is
            # drained at the TOP of each iteration so it overlaps that slab's
            # matmuls, leaving only slab 15's evict_b after the last matmul.
            # ypool ring safety: live ytile slabs stay <= 8 = 32/SUB, and the
            # freeing evict_b is always emitted before the reusing evict_a.
            done_b = 0
            for s in range(1, NS):
                for kk in slab_mms(s, [nc.sync, nc.scalar]):
                    pass
                want = s - 6 if s < 10 else 2 * (s - 10) + 5
                while done_b <= min(want, s - 1):
                    evict_b(done_b)
                    done_b += 1
                evict_a(s)
                if s <= 4:
                    for i in range(4 * s - 2, min(4 * s + 2, 16)):
                        s_reduce(i)
                    if s == 4:
                        s_finish()
            while done_b < NS:
                evict_b(done_b)
                done_b += 1

    nc.compile()
    return nc


def kernel(x, q_values, q_scales, l_values, l_scales, r_values, r_scales, bias,
           _trace=False):
    from concourse.bass_utils import run_bass_kernel_spmd

    bf16 = ml_dtypes.bfloat16
    e4m3 = ml_dtypes.float8_e4m3

    if "m" not in _module_cache:
        _module_cache["m"] = _build_module()
    nc = _module_cache["m"]

    # host-side marshaling (layout + dtype + power-of-two scaling only)
    x = np.asarray(x, dtype=np.float32)
    q_values = np.asarray(q_values)
    q_scales = np.asarray(q_scales, np.float32)
    l_values = np.asarray(l_values)
    l_scales = np.asarray(l_scales, np.float32)
    r_values = np.asarray(r_values)
    r_scales = np.asarray(r_scales, np.float32)
    bias = np.asarray(bias, np.float32)

    # x*16 as e4m3, tiled [NS, KKT, P, 2, MS]: plane ko holds i = kk*256+ko*128+p
    xs = np.clip(x * 16.0, -240.0, 240.0)
    xt8 = np.ascontiguousarray(
        xs.reshape(NS, MS, KKT, 2, P).transpose(0, 2, 4, 3, 1)
    ).astype(e4m3)
    # bf16 raw-row x for the S reduction, per-core shard below
    xb_all = x.astype(bf16)

    rs_full = np.repeat(r_scales, D_IN // r_scales.shape[1], axis=1)
    rv_f = r_values.astype(np.float32)
    # wbr[p, ch, j, 0, :] = r codes, [.., 1, :] = broadcast r scales
    wbr = np.empty((P, RCH, RG, 2, RCW), np.float32)
    for ch in range(RCH):
        cs = slice(ch * RCW, (ch + 1) * RCW)
        for j in range(RG):
            wbr[:, ch, j, 0, :] = rv_f[j * P:(j + 1) * P, cs]
            wbr[:, ch, j, 1, :] = rs_full[j * P:(j + 1) * P, cs]
    wbr = wbr.astype(e4m3)

    in_maps = []
    for c in range(NCORES):
        sl = slice(c * OC, (c + 1) * OC)
        qt_c = q_values[sl].T.astype(np.float32)            # [D_IN, OC]
        qst_c = (q_scales[sl].T / 16.0).astype(np.float32)  # [KT, OC]
        ltv_c = l_values[sl].T.astype(np.float32)           # [RANK, OC]
        lst_c = (l_scales[sl].T / 16.0).astype(np.float32)  # [RG, OC]

        lcod = np.empty((P, RG, OC), np.float32)
        for j in range(RG):
            lcod[:, j, :] = ltv_c[j * P:(j + 1) * P, :]
        wbq = np.empty((P, KT, 2, OC), np.float32)
        for k in range(KT):
            wbq[:, k, 0, :] = qt_c[k * P:(k + 1) * P, :]
            wbq[:, k, 1, :] = np.broadcast_to(qst_c[k], (P, OC))

        in_maps.append({
            "xt": xt8,
            "lcod": lcod.astype(e4m3),
            "lsc": np.ascontiguousarray(lst_c.reshape(1, RG, OC)).astype(bf16),
            "wbr": wbr,
            "wbq": wbq.astype(e4m3),
            "xb": np.ascontiguousarray(
                xb_all[c * SHTOK:(c + 1) * SHTOK].reshape(SHT, P, 2, D_IN // 2)),
            "biasr": np.ascontiguousarray(bias[sl].reshape(1, OC)).astype(bf16),
        })

    res = run_bass_kernel_spmd(
        nc, in_maps, core_ids=list(range(NCORES)), trace=_trace
    )
    global last_result
    last_result = res
    return np.concatenate([r["y"] for r in res.results], axis=1)


# revision 84
# speedup vs baseline: 1.0519x; 1.0067x over previous
"""CalderaLinear fused kernel for 8 Trainium2 NeuronCores — fp8 DoubleRow.

Math (reference): y = x @ Q^T + (x @ R^T) @ L^T + bias, with Q/L/R groupwise
int-dequantized (codes 0..15, group size 128).

Strategy (v2, fp8):
  * Column-parallel over d_out: core c owns out-features [c*512, (c+1)*512).
  * W_c = Q_c^T + R^T L_c^T ([d_in, 512]) has column means ~3600 (the R^T L^T
    product of non-negative codes) while the fluctuation around the mean is
    only ~270 rms.  fp8 e4m3's 3-bit mantissa on raw W gives ~2.5e-2 rel
    error (fails the 2e-2 gate), but on the *centered* W it gives ~4e-3.
    So the kernel computes, all on device:
        m_r   = mean_i r_deq[r, i]                    (DVE reduce)
        What  = Q^T/16 + (R - 1 m)^T (L/16)^T          (PE + DVE, cast e4m3)
        mu_o  = 16 * m @ (L/16)^T                      (PE, rank-1 weights)
        S_t   = sum_i x[t, i]                          (DVE reduce over bf16 x)
        y     = (16x)_fp8 @ What_fp8 + S * mu + bias   (PE DoubleRow + DVE/ACT)
    The rank-1 S*mu term restores the removed mean exactly; the fp8 rounding
    only ever touches the small fluctuating part.  Simulated rel_l2 ~4e-3.
  * The main matmul runs in MatmulPerfMode.DoubleRow: both operands e4m3,
    3D APs [128, 2, free] carrying two contraction planes per partition,
    contraction 256 per matmul -> half the matmul count of bf16.
  * S is data-parallel: core c reduces its own 1024 token rows (raw-layout
    bf16 x shard), then a 4 KB AllGather shares all 8192 sums.  Evictions
    are split: psum+bias -> SBUF immediately (frees PSUM), the +S*mu
    correction + store lag one slab so the collective latency hides.
  * x streams as pre-tiled fp8 [128, 2, 512] blocks; W stays SBUF-resident
    fp8; PSUM accumulates over the 16 doubled k-tiles; bias fused into
    eviction.  Host side only reshapes/transposes/casts/scales-by-2^±4 and
    concatenates the 8 output shards: all dequant + matmul + reduction math
    runs on the NeuronCores.
"""

import numpy as np
import ml_dtypes

P = 128
D_IN = 4096
D_OUT = 4096
TOK = 8192
RANK = 256
NCORES = 8
OC = D_OUT // NCORES      # 512 out features per core
KT = D_IN // P            # 32 contraction tiles (bf16 build granularity)
KKT = KT // 2             # 16 doubled contraction tiles (fp8 DoubleRow)
MS = 512                  # token slab
NS = TOK // MS            # 16 slabs
SUB = MS // P             # 4 psum sub-tiles per slab
RG = RANK // 128          # 2 rank tiles
RCH = 8                   # R chunks along d_in
RCW = D_IN // RCH         # 512 R columns per chunk
SHTOK = TOK // NCORES     # 1024 tokens per core for the S reduction
SHT = SHTOK // P          # 8 row-tiles of the S shard

_module_cache = {}
last_result = None


def _build_module():
    import concourse.mybir as mybir
    import concourse.tile as tile
    from concourse import bacc

    bf = mybir.dt.bfloat16
    f8 = mybir.dt.float8e4
    f32 = mybir.dt.float32
    AX = mybir.AxisListType
    DR = mybir.MatmulPerfMode.DoubleRow
    COPY = mybir.ActivationFunctionType.Copy

    nc = bacc.Bacc(None, target_bir_lowering=False, debug=False,
                   num_devices=NCORES)
    xt_d = nc.dram_tensor("xt", (NS, KKT, P, 2, MS), f8, kind="ExternalInput")
    lcod_d = nc.dram_tensor("lcod", (P, RG, OC), f8, kind="ExternalInput")
    lsc_d = nc.dram_tensor("lsc", (1, RG, OC), bf, kind="ExternalInput")
    wbr_d = nc.dram_tensor("wbr", (P, RCH, RG, 2, RCW), f8, kind="ExternalInput")
    wbq_d = nc.dram_tensor("wbq", (P, KT, 2, OC), f8, kind="ExternalInput")
    XBH = D_IN // 2
    xb_d = nc.dram_tensor("xb", (SHT, P, 2, XBH), bf, kind="ExternalInput")
    biasr_d = nc.dram_tensor("biasr", (1, OC), bf, kind="ExternalInput")
    y_d = nc.dram_tensor("y", (TOK, OC), f32, kind="ExternalOutput")

    with tile.TileContext(nc) as tc:
        with (
            tc.tile_pool(name="const", bufs=1) as const,
            tc.tile_pool(name="wpool", bufs=1) as wpool,
            tc.tile_pool(name="xpool", bufs=16) as xpool,
            tc.tile_pool(name="xbpool", bufs=8) as xbpool,
            tc.tile_pool(name="qpool", bufs=4) as qpool,
            tc.tile_pool(name="ypool", bufs=28) as ypool,
            tc.tile_pool(name="cpool", bufs=4) as cpool,
            tc.tile_pool(name="ppool", bufs=8, space="PSUM") as ppool,
            tc.tile_pool(name="dpool", bufs=1, space="DRAM") as dpool,
        ):
            sin_d = dpool.tile([P, SHT], f32, name="sin")
            sout_d = dpool.tile([NCORES, P, SHT], f32, name="sout",
                                addr_space="Shared")
            Lcod = const.tile([P, RG, OC], f8)
            Lscr = const.tile([1, RG, OC], bf)
            biasr = const.tile([1, OC], bf)
            WBR = const.tile([P, RCH, RG, 2, RCW], f8)
            RD = const.tile([P, RCH, RG, RCW], bf)
            bias_t = const.tile([P, OC], f32)
            LdT = const.tile([P, RG, OC], bf)
            Wt = wpool.tile([P, KKT, 2, OC], f8)
            mneg = const.tile([P, RG], f32)     # -mean est. (chunk 0 cols)
            m16 = const.tile([P, RG], bf)       # +m estimate (bf16, mu lhsT)
            wu = const.tile([P, OC], bf)        # warmup scratch
            ones1 = const.tile([1, P], bf)
            murow = const.tile([1, OC], bf)
            munegrow = const.tile([1, OC], bf)
            mub = const.tile([P, OC], f32)      # broadcast 16*mu/16 = mu
            S_sb = const.tile([P, NS * SUB], f32)
            Sown = const.tile([P, SHT], f32)
            Spart = const.tile([P, SHT, 2], f32)

            # ---- phase-0 DMAs.  sync: R chunk 0 + L header + bias row
            # (small, feeds the critical m~/W-build chain); scalar: chunk 1;
            # gpsimd: R chunks 2-7, then the bf16 x shard for S.
            nc.sync.dma_start(WBR[:, 0], wbr_d[:, 0])
            nc.scalar.dma_start(WBR[:, 1], wbr_d[:, 1])
            nc.sync.dma_start(Lcod[:], lcod_d[:])
            nc.sync.dma_start(Lscr[:], lsc_d[:])
            nc.sync.dma_start(biasr[:], biasr_d[:])
            for ch in range(2, RCH):
                nc.gpsimd.dma_start(WBR[:, ch], wbr_d[:, ch])
            xbt = []
            for j in range(SHT):
                for h in range(2):
                    t = xbpool.tile([P, XBH], bf, tag="xb", name=f"xb{j}_{h}")
                    nc.gpsimd.dma_start(t[:], xb_d[j, :, h])
                    xbt.append(t)

            # ---- PE warmup: HAM un-throttles after ~3.4us of activity, so
            # burn idle DMA-wait time on dummy matmuls at the very start.
            nc.vector.memset(wu[:], 0.0)
            wu_ps = ppool.tile([P, OC], f32, tag="ps", name="wups")
            for i in range(18):
                nc.tensor.matmul(wu_ps[:], wu[:, 0:P], wu[:],
                                 start=True, stop=True)

            nc.vector.memset(ones1[:], 1.0)

            # ---- L^T dequant: broadcast the [1, OC] scale rows to all 128
            # partitions with a K=1 ones-matmul, then codes x scales on DVE
            sc_ps = []
            for j in range(RG):
                ps = ppool.tile([P, OC], f32, tag="ps", name=f"lsc{j}")
                nc.tensor.matmul(ps[:], ones1[:], Lscr[:, j, :],
                                 start=True, stop=True)
                sc_ps.append(ps)
            for j in range(RG):
                nc.vector.tensor_mul(LdT[:, j, :], Lcod[:, j, :],
                                     sc_ps[j][:])

            # ---- mean estimate from R chunk 0 only.  The centering identity
            # x@(W - 1 m~ L16) + S*(m~ L16) == x@W holds exactly for ANY m~;
            # a 512-column estimate only leaves a negligible rank-1 residual
            # in the fp8 rounding, and it kills the full-R DMA dependency.
            def deq_center(ch):
                for j in range(RG):
                    nc.vector.tensor_mul(RD[:, ch, j, :],
                                         WBR[:, ch, j, 0, :],
                                         WBR[:, ch, j, 1, :])
                    nc.vector.tensor_scalar_add(RD[:, ch, j, :],
                                                RD[:, ch, j, :],
                                                mneg[:, j:j + 1])

            for j in range(RG):
                nc.vector.tensor_mul(RD[:, 0, j, :], WBR[:, 0, j, 0, :],
                                     WBR[:, 0, j, 1, :])
                nc.vector.reduce_sum(mneg[:, j:j + 1], RD[:, 0, j, :],
                                     axis=AX.X, negate=True)
            nc.vector.tensor_scalar_mul(m16[:], mneg[:], -1.0 / RCW)
            nc.vector.tensor_scalar_mul(mneg[:], mneg[:], 1.0 / RCW)
            for j in range(RG):
                nc.vector.tensor_scalar_add(RD[:, 0, j, :], RD[:, 0, j, :],
                                            mneg[:, j:j + 1])

            # ---- mu = 16 * m @ (L/16)^T, broadcast to all 128 partitions
            mu_ps = ppool.tile([P, OC], f32, tag="ps", name="mu1")
            for j in range(RG):
                nc.tensor.matmul(mu_ps[0:1, :], m16[:, j:j + 1], LdT[:, j, :],
                                 start=(j == 0), stop=(j == RG - 1))
            nc.vector.tensor_copy(murow[:], mu_ps[0:1, :])
            nc.vector.tensor_scalar_mul(munegrow[:], mu_ps[0:1, :], -1.0)
            mub_ps = ppool.tile([P, OC], f32, tag="ps", name="mu2")
            nc.tensor.matmul(mub_ps[:], ones1[:], murow[:], start=True,
                             stop=True)
            nc.vector.tensor_scalar_mul(mub[:], mub_ps[:], 16.0)

            # ---- bias broadcast [1, OC] -> [128, OC] the same way
            bb_ps = ppool.tile([P, OC], f32, tag="ps", name="bb")
            nc.tensor.matmul(bb_ps[:], ones1[:], biasr[:], start=True,
                             stop=True)
            nc.vector.tensor_copy(bias_t[:], bb_ps[:])

            # ---- W build: What k-tile k = R_hat^T(L/16)^T + Q^T/16 -> e4m3
            def rd(j, k):
                return RD[:, k // 4, j, (k % 4) * P:(k % 4) * P + P]

            def build_w(k):
                ps = ppool.tile([P, OC], f32, tag="ps", name=f"wb{k}")
                for j in range(RG):
                    nc.tensor.matmul(ps[:], rd(j, k), LdT[:, j, :],
                                     start=(j == 0), stop=(j == RG - 1))
                qt = qpool.tile([P, 2, OC], f8, tag="qt")
                eng = nc.sync if k % 2 == 0 else nc.scalar
                eng.dma_start(qt[:], wbq_d[:, k])
                nc.vector.tensor_mul(qt[:, 0, :], qt[:, 0, :], qt[:, 1, :])
                nc.vector.tensor_add(Wt[:, k // 2, k % 2, :], ps[:],
                                     qt[:, 0, :])

            psums = {}

            def slab_mms(s, dma_engines):
                psums[s] = [ppool.tile([P, OC], f32, tag="ps",
                                       name=f"ps{s}_{i}") for i in range(SUB)]
                for kk in range(KKT):
                    xt = xpool.tile([P, 2, MS], f8, tag="x")
                    dma_engines[kk % len(dma_engines)].dma_start(
                        xt[:], xt_d[s, kk])
                    for sub in range(SUB):
                        nc.tensor.matmul(
                            psums[s][sub][:],
                            xt[:, :, sub * P:(sub + 1) * P],
                            Wt[:, kk, :, :],
                            start=(kk == 0), stop=(kk == KKT - 1),
                            perf_mode=DR,
                        )
                    yield kk

            ytiles = {}

            def evict_a(s):
                # psum + bias -> SBUF, frees the psum bank
                ytiles[s] = []
                for sub in range(SUB):
                    yt = ypool.tile([P, OC], f32, tag="y", name=f"y{s}_{sub}")
                    nc.vector.tensor_add(yt[:], psums[s][sub][:], bias_t[:])
                    ytiles[s].append(yt)

            def evict_b(s):
                # + S*mu (ACT outer product), then store
                for sub in range(SUB):
                    col = s * SUB + sub
                    corr = cpool.tile([P, OC], f32, tag="c")
                    nc.scalar.activation(corr[:], mub[:], COPY,
                                         scale=S_sb[:, col:col + 1])
                    yt = ytiles[s][sub]
                    nc.vector.tensor_add(yt[:], yt[:], corr[:])
                    # y stores ride the gpsimd queue (idle after the x shard),
                    # keeping sync/scalar free for the x8 stream; the last two
                    # slabs fan out across all three queues to shrink the tail
                    eng = nc.gpsimd if s < NS - 1 else (
                        nc.scalar if sub % 2 == 0 else nc.sync)
                    eng.dma_start(
                        y_d[s * MS + sub * P:s * MS + (sub + 1) * P, :], yt[:])

            # ---- S shard: reduce own 1024 bf16 token rows (spread across
            # slabs 1-4 so psum eviction never waits), AllGather 4 KB
            def s_reduce(i):
                h = i % 2
                nc.vector.reduce_sum(Spart[:, i // 2, h:h + 1], xbt[i][:],
                                     axis=AX.X)

            def s_finish():
                nc.vector.reduce_sum(Sown[:], Spart[:], axis=AX.X)
                nc.gpsimd.dma_start(sin_d[:], Sown[:])
                nc.gpsimd.collective_compute(
                    "AllGather",
                    mybir.AluOpType.bypass,
                    replica_groups=[list(range(NCORES))],
                    ins=[sin_d.opt()],
                    outs=[sout_d.opt()],
                )
                for c in range(NCORES):
                    nc.gpsimd.dma_start(S_sb[:, c * SHT:(c + 1) * SHT],
                                        sout_d[c])

            # ---- slab 0: W build interleaved two tiles ahead, R chunks
            # dequantized + centered just-in-time as their DMAs land
            for k in range(4):
                build_w(k)
            for kk in slab_mms(0, [nc.sync, nc.scalar]):
                if kk % 2 == 0 and kk // 2 + 1 < RCH:
                    deq_center(kk // 2 + 1)
                if kk >= 12 and kk % 2 == 0:
                    s_reduce(kk // 2 - 6)
                if kk < KKT - 2:
                    build_w(2 * kk + 4)
                    build_w(2 * kk + 5)

            evict_a(0)

            # evict_b lags ~7 slabs mid-run (rides out the S collective
            # latency), catches up two-per-slab from slab 10.  The backlog is
            # drained at the TOP of each iteration so it overlaps that slab's
            # matmuls, leaving only slab 15's evict_b after the last matmul.
            # ypool ring safety: live ytile slabs stay <= 8 = 32/SUB, and the
            # freeing evict_b is always emitted before the reusing evict_a.
            done_b = 0
            for s in range(1, NS):
                for kk in slab_mms(s, [nc.sync, nc.scalar]):
                    pass
                want = s - 6 if s < 10 else 2 * (s - 10) + 5
                while done_b <= min(want, s - 1):
                    evict_b(done_b)
                    done_b += 1
                evict_a(s)
                if s <= 4:
                    for i in range(4 * s - 2, min(4 * s + 2, 16)):
                        s_reduce(i)
                    if s == 4:
                        s_finish()
            while done_b < NS:
                evict_b(done_b)
                done_b += 1

    nc.compile()
    return nc


def kernel(x, q_values, q_scales, l_values, l_scales, r_values, r_scales, bias,
           _trace=False):
    from concourse.bass_utils import run_bass_kernel_spmd

    bf16 = ml_dtypes.bfloat16
    e4m3 = ml_dtypes.float8_e4m3

    if "m" not in _module_cache:
        _module_cache["m"] = _build_module()
    nc = _module_cache["m"]

    # host-side marshaling (layout + dtype + power-of-two scaling only)
    x = np.asarray(x, dtype=np.float32)
    q_values = np.asarray(q_values)
    q_scales = np.asarray(q_scales, np.float32)
    l_values = np.asarray(l_values)
    l_scales = np.asarray(l_scales, np.float32)
    r_values = np.asarray(r_values)
    r_scales = np.asarray(r_scales, np.float32)
    bias = np.asarray(bias, np.float32)

    # x*16 as e4m3, tiled [NS, KKT, P, 2, MS]: plane ko holds i = kk*256+ko*128+p
    xs = np.clip(x * 16.0, -240.0, 240.0)
    xt8 = np.ascontiguousarray(
        xs.reshape(NS, MS, KKT, 2, P).transpose(0, 2, 4, 3, 1)
    ).astype(e4m3)
    # bf16 raw-row x for the S reduction, per-core shard below
    xb_all = x.astype(bf16)

    rs_full = np.repeat(r_scales, D_IN // r_scales.shape[1], axis=1)
    rv_f = r_values.astype(np.float32)
    # wbr[p, ch, j, 0, :] = r codes, [.., 1, :] = broadcast r scales
    wbr = np.empty((P, RCH, RG, 2, RCW), np.float32)
    for ch in range(RCH):
        cs = slice(ch * RCW, (ch + 1) * RCW)
        for j in range(RG):
            wbr[:, ch, j, 0, :] = rv_f[j * P:(j + 1) * P, cs]
            wbr[:, ch, j, 1, :] = rs_full[j * P:(j + 1) * P, cs]
    wbr = wbr.astype(e4m3)

    in_maps = []
    for c in range(NCORES):
        sl = slice(c * OC, (c + 1) * OC)
        qt_c = q_values[sl].T.astype(np.float32)            # [D_IN, OC]
        qst_c = (q_scales[sl].T / 16.0).astype(np.float32)  # [KT, OC]
        ltv_c = l_values[sl].T.astype(np.float32)           # [RANK, OC]
        lst_c = (l_scales[sl].T / 16.0).astype(np.float32)  # [RG, OC]

        lcod = np.empty((P, RG, OC), np.float32)
        for j in range(RG):
            lcod[:, j, :] = ltv_c[j * P:(j + 1) * P, :]
        wbq = np.empty((P, KT, 2, OC), np.float32)
        for k in range(KT):
            wbq[:, k, 0, :] = qt_c[k * P:(k + 1) * P, :]
            wbq[:, k, 1, :] = np.broadcast_to(qst_c[k], (P, OC))

        in_maps.append({
            "xt": xt8,
            "lcod": lcod.astype(e4m3),
            "lsc": np.ascontiguousarray(lst_c.reshape(1, RG, OC)).astype(bf16),
            "wbr": wbr,
            "wbq": wbq.astype(e4m3),
            "xb": np.ascontiguousarray(
                xb_all[c * SHTOK:(c + 1) * SHTOK].reshape(SHT, P, 2, D_IN // 2)),
            "biasr": np.ascontiguousarray(bias[sl].reshape(1, OC)).astype(bf16),
        })

    res = run_bass_kernel_spmd(
        nc, in_maps, core_ids=list(range(NCORES)), trace=_trace
    )
    global last_result
    last_result = res
    return np.concatenate([r["y"] for r in res.results], axis=1)


# revision 85
# speedup vs baseline: 1.0581x; 1.0059x over previous
"""CalderaLinear fused kernel for 8 Trainium2 NeuronCores — fp8 DoubleRow.

Math (reference): y = x @ Q^T + (x @ R^T) @ L^T + bias, with Q/L/R groupwise
int-dequantized (codes 0..15, group size 128).

Strategy (v2, fp8):
  * Column-parallel over d_out: core c owns out-features [c*512, (c+1)*512).
  * W_c = Q_c^T + R^T L_c^T ([d_in, 512]) has column means ~3600 (the R^T L^T
    product of non-negative codes) while the fluctuation around the mean is
    only ~270 rms.  fp8 e4m3's 3-bit mantissa on raw W gives ~2.5e-2 rel
    error (fails the 2e-2 gate), but on the *centered* W it gives ~4e-3.
    So the kernel computes, all on device:
        m_r   = mean_i r_deq[r, i]                    (DVE reduce)
        What  = Q^T/16 + (R - 1 m)^T (L/16)^T          (PE + DVE, cast e4m3)
        mu_o  = 16 * m @ (L/16)^T                      (PE, rank-1 weights)
        S_t   = sum_i x[t, i]                          (DVE reduce over bf16 x)
        y     = (16x)_fp8 @ What_fp8 + S * mu + bias   (PE DoubleRow + DVE/ACT)
    The rank-1 S*mu term restores the removed mean exactly; the fp8 rounding
    only ever touches the small fluctuating part.  Simulated rel_l2 ~4e-3.
  * The main matmul runs in MatmulPerfMode.DoubleRow: both operands e4m3,
    3D APs [128, 2, free] carrying two contraction planes per partition,
    contraction 256 per matmul -> half the matmul count of bf16.
  * S is data-parallel: core c reduces its own 1024 token rows (raw-layout
    bf16 x shard), then a 4 KB AllGather shares all 8192 sums.  Evictions
    are split: psum+bias -> SBUF immediately (frees PSUM), the +S*mu
    correction + store lag one slab so the collective latency hides.
  * x streams as pre-tiled fp8 [128, 2, 512] blocks; W stays SBUF-resident
    fp8; PSUM accumulates over the 16 doubled k-tiles; bias fused into
    eviction.  Host side only reshapes/transposes/casts/scales-by-2^±4 and
    concatenates the 8 output shards: all dequant + matmul + reduction math
    runs on the NeuronCores.
"""

import numpy as np
import ml_dtypes

P = 128
D_IN = 4096
D_OUT = 4096
TOK = 8192
RANK = 256
NCORES = 8
OC = D_OUT // NCORES      # 512 out features per core
KT = D_IN // P            # 32 contraction tiles (bf16 build granularity)
KKT = KT // 2             # 16 doubled contraction tiles (fp8 DoubleRow)
MS = 512                  # token slab
NS = TOK // MS            # 16 slabs
SUB = MS // P             # 4 psum sub-tiles per slab
RG = RANK // 128          # 2 rank tiles
RCH = 8                   # R chunks along d_in
RCW = D_IN // RCH         # 512 R columns per chunk
SHTOK = TOK // NCORES     # 1024 tokens per core for the S reduction
SHT = SHTOK // P          # 8 row-tiles of the S shard

_module_cache = {}
last_result = None


def _build_module():
    import concourse.mybir as mybir
    import concourse.tile as tile
    from concourse import bacc

    bf = mybir.dt.bfloat16
    f8 = mybir.dt.float8e4
    f32 = mybir.dt.float32
    AX = mybir.AxisListType
    DR = mybir.MatmulPerfMode.DoubleRow
    COPY = mybir.ActivationFunctionType.Copy

    nc = bacc.Bacc(None, target_bir_lowering=False, debug=False,
                   num_devices=NCORES)
    xt_d = nc.dram_tensor("xt", (NS, KKT, P, 2, MS), f8, kind="ExternalInput")
    lcod_d = nc.dram_tensor("lcod", (P, RG, OC), f8, kind="ExternalInput")
    lsc_d = nc.dram_tensor("lsc", (1, RG, OC), bf, kind="ExternalInput")
    wbr_d = nc.dram_tensor("wbr", (P, RCH, RG, 2, RCW), f8, kind="ExternalInput")
    wbq_d = nc.dram_tensor("wbq", (P, KT, 2, OC), f8, kind="ExternalInput")
    XBH = D_IN // 2
    xb_d = nc.dram_tensor("xb", (SHT, P, 2, XBH), bf, kind="ExternalInput")
    biasr_d = nc.dram_tensor("biasr", (1, OC), bf, kind="ExternalInput")
    y_d = nc.dram_tensor("y", (TOK, OC), f32, kind="ExternalOutput")

    with tile.TileContext(nc) as tc:
        with (
            tc.tile_pool(name="const", bufs=1) as const,
            tc.tile_pool(name="wpool", bufs=1) as wpool,
            tc.tile_pool(name="xpool", bufs=20) as xpool,
            tc.tile_pool(name="xbpool", bufs=8) as xbpool,
            tc.tile_pool(name="qpool", bufs=8) as qpool,
            tc.tile_pool(name="ypool", bufs=28) as ypool,
            tc.tile_pool(name="cpool", bufs=4) as cpool,
            tc.tile_pool(name="ppool", bufs=8, space="PSUM") as ppool,
            tc.tile_pool(name="dpool", bufs=1, space="DRAM") as dpool,
        ):
            sin_d = dpool.tile([P, SHT], f32, name="sin")
            sout_d = dpool.tile([NCORES, P, SHT], f32, name="sout",
                                addr_space="Shared")
            Lcod = const.tile([P, RG, OC], f8)
            Lscr = const.tile([1, RG, OC], bf)
            biasr = const.tile([1, OC], bf)
            WBR = const.tile([P, RCH, RG, 2, RCW], f8)
            RD = const.tile([P, RCH, RG, RCW], bf)
            bias_t = const.tile([P, OC], f32)
            LdT = const.tile([P, RG, OC], bf)
            Wt = wpool.tile([P, KKT, 2, OC], f8)
            mneg = const.tile([P, RG], f32)     # -mean est. (chunk 0 cols)
            m16 = const.tile([P, RG], bf)       # +m estimate (bf16, mu lhsT)
            wu = const.tile([P, OC], bf)        # warmup scratch
            ones1 = const.tile([1, P], bf)
            murow = const.tile([1, OC], bf)
            munegrow = const.tile([1, OC], bf)
            mub = const.tile([P, OC], f32)      # broadcast 16*mu/16 = mu
            S_sb = const.tile([P, NS * SUB], f32)
            Sown = const.tile([P, SHT], f32)
            Spart = const.tile([P, SHT, 2], f32)

            # ---- phase-0 DMAs.  sync: R chunk 0 + L header + bias row
            # (small, feeds the critical m~/W-build chain); scalar: chunk 1;
            # gpsimd: R chunks 2-7, then the bf16 x shard for S.
            nc.sync.dma_start(WBR[:, 0], wbr_d[:, 0])
            nc.scalar.dma_start(WBR[:, 1], wbr_d[:, 1])
            nc.sync.dma_start(Lcod[:], lcod_d[:])
            nc.sync.dma_start(Lscr[:], lsc_d[:])
            nc.sync.dma_start(biasr[:], biasr_d[:])
            for ch in range(2, RCH):
                nc.gpsimd.dma_start(WBR[:, ch], wbr_d[:, ch])
            xbt = []
            for j in range(SHT):
                for h in range(2):
                    t = xbpool.tile([P, XBH], bf, tag="xb", name=f"xb{j}_{h}")
                    nc.gpsimd.dma_start(t[:], xb_d[j, :, h])
                    xbt.append(t)

            # ---- PE warmup: HAM un-throttles after ~3.4us of activity, so
            # burn idle DMA-wait time on dummy matmuls at the very start.
            nc.vector.memset(wu[:], 0.0)
            wu_ps = ppool.tile([P, OC], f32, tag="ps", name="wups")
            for i in range(18):
                nc.tensor.matmul(wu_ps[:], wu[:, 0:P], wu[:],
                                 start=True, stop=True)

            nc.vector.memset(ones1[:], 1.0)

            # ---- L^T dequant: broadcast the [1, OC] scale rows to all 128
            # partitions with a K=1 ones-matmul, then codes x scales on DVE
            sc_ps = []
            for j in range(RG):
                ps = ppool.tile([P, OC], f32, tag="ps", name=f"lsc{j}")
                nc.tensor.matmul(ps[:], ones1[:], Lscr[:, j, :],
                                 start=True, stop=True)
                sc_ps.append(ps)
            for j in range(RG):
                nc.vector.tensor_mul(LdT[:, j, :], Lcod[:, j, :],
                                     sc_ps[j][:])

            # ---- mean estimate from R chunk 0 only.  The centering identity
            # x@(W - 1 m~ L16) + S*(m~ L16) == x@W holds exactly for ANY m~;
            # a 512-column estimate only leaves a negligible rank-1 residual
            # in the fp8 rounding, and it kills the full-R DMA dependency.
            def deq_center(ch):
                for j in range(RG):
                    nc.vector.tensor_mul(RD[:, ch, j, :],
                                         WBR[:, ch, j, 0, :],
                                         WBR[:, ch, j, 1, :])
                    nc.vector.tensor_scalar_add(RD[:, ch, j, :],
                                                RD[:, ch, j, :],
                                                mneg[:, j:j + 1])

            for j in range(RG):
                nc.vector.tensor_mul(RD[:, 0, j, :], WBR[:, 0, j, 0, :],
                                     WBR[:, 0, j, 1, :])
                nc.vector.reduce_sum(mneg[:, j:j + 1], RD[:, 0, j, :],
                                     axis=AX.X, negate=True)
            nc.vector.tensor_scalar_mul(m16[:], mneg[:], -1.0 / RCW)
            nc.vector.tensor_scalar_mul(mneg[:], mneg[:], 1.0 / RCW)
            for j in range(RG):
                nc.vector.tensor_scalar_add(RD[:, 0, j, :], RD[:, 0, j, :],
                                            mneg[:, j:j + 1])

            # ---- mu = 16 * m @ (L/16)^T, broadcast to all 128 partitions
            mu_ps = ppool.tile([P, OC], f32, tag="ps", name="mu1")
            for j in range(RG):
                nc.tensor.matmul(mu_ps[0:1, :], m16[:, j:j + 1], LdT[:, j, :],
                                 start=(j == 0), stop=(j == RG - 1))
            nc.vector.tensor_copy(murow[:], mu_ps[0:1, :])
            nc.vector.tensor_scalar_mul(munegrow[:], mu_ps[0:1, :], -1.0)
            mub_ps = ppool.tile([P, OC], f32, tag="ps", name="mu2")
            nc.tensor.matmul(mub_ps[:], ones1[:], murow[:], start=True,
                             stop=True)
            nc.vector.tensor_scalar_mul(mub[:], mub_ps[:], 16.0)

            # ---- bias broadcast [1, OC] -> [128, OC] the same way
            bb_ps = ppool.tile([P, OC], f32, tag="ps", name="bb")
            nc.tensor.matmul(bb_ps[:], ones1[:], biasr[:], start=True,
                             stop=True)
            nc.vector.tensor_copy(bias_t[:], bb_ps[:])

            # ---- W build: What k-tile k = R_hat^T(L/16)^T + Q^T/16 -> e4m3
            def rd(j, k):
                return RD[:, k // 4, j, (k % 4) * P:(k % 4) * P + P]

            def build_w(k):
                ps = ppool.tile([P, OC], f32, tag="ps", name=f"wb{k}")
                for j in range(RG):
                    nc.tensor.matmul(ps[:], rd(j, k), LdT[:, j, :],
                                     start=(j == 0), stop=(j == RG - 1))
                qt = qpool.tile([P, 2, OC], f8, tag="qt")
                eng = nc.sync if k % 2 == 0 else nc.scalar
                eng.dma_start(qt[:], wbq_d[:, k])
                nc.vector.tensor_mul(qt[:, 0, :], qt[:, 0, :], qt[:, 1, :])
                nc.vector.tensor_add(Wt[:, k // 2, k % 2, :], ps[:],
                                     qt[:, 0, :])

            psums = {}

            def slab_mms(s, dma_engines):
                psums[s] = [ppool.tile([P, OC], f32, tag="ps",
                                       name=f"ps{s}_{i}") for i in range(SUB)]
                for kk in range(KKT):
                    xt = xpool.tile([P, 2, MS], f8, tag="x")
                    dma_engines[kk % len(dma_engines)].dma_start(
                        xt[:], xt_d[s, kk])
                    for sub in range(SUB):
                        nc.tensor.matmul(
                            psums[s][sub][:],
                            xt[:, :, sub * P:(sub + 1) * P],
                            Wt[:, kk, :, :],
                            start=(kk == 0), stop=(kk == KKT - 1),
                            perf_mode=DR,
                        )
                    yield kk

            ytiles = {}

            def evict_a(s):
                # psum + bias -> SBUF, frees the psum bank
                ytiles[s] = []
                for sub in range(SUB):
                    yt = ypool.tile([P, OC], f32, tag="y", name=f"y{s}_{sub}")
                    nc.vector.tensor_add(yt[:], psums[s][sub][:], bias_t[:])
                    ytiles[s].append(yt)

            def evict_b(s):
                # + S*mu (ACT outer product), then store
                for sub in range(SUB):
                    col = s * SUB + sub
                    corr = cpool.tile([P, OC], f32, tag="c")
                    nc.scalar.activation(corr[:], mub[:], COPY,
                                         scale=S_sb[:, col:col + 1])
                    yt = ytiles[s][sub]
                    nc.vector.tensor_add(yt[:], yt[:], corr[:])
                    # y stores ride the gpsimd queue (idle after the x shard),
                    # keeping sync/scalar free for the x8 stream; the last two
                    # slabs fan out across all three queues to shrink the tail
                    eng = nc.gpsimd if s < NS - 1 else (
                        nc.scalar if sub % 2 == 0 else nc.sync)
                    eng.dma_start(
                        y_d[s * MS + sub * P:s * MS + (sub + 1) * P, :], yt[:])

            # ---- S shard: reduce own 1024 bf16 token rows (spread across
            # slabs 1-4 so psum eviction never waits), AllGather 4 KB
            def s_reduce(i):
                h = i % 2
                nc.vector.reduce_sum(Spart[:, i // 2, h:h + 1], xbt[i][:],
                                     axis=AX.X)

            def s_finish():
                nc.vector.reduce_sum(Sown[:], Spart[:], axis=AX.X)
                nc.gpsimd.dma_start(sin_d[:], Sown[:])
                nc.gpsimd.collective_compute(
                    "AllGather",
                    mybir.AluOpType.bypass,
                    replica_groups=[list(range(NCORES))],
                    ins=[sin_d.opt()],
                    outs=[sout_d.opt()],
                )
                for c in range(NCORES):
                    nc.gpsimd.dma_start(S_sb[:, c * SHT:(c + 1) * SHT],
                                        sout_d[c])

            # ---- slab 0: W build interleaved two tiles ahead, R chunks
            # dequantized + centered just-in-time as their DMAs land
            for k in range(4):
                build_w(k)
            for kk in slab_mms(0, [nc.sync, nc.scalar]):
                if kk % 2 == 0 and kk // 2 + 1 < RCH:
                    deq_center(kk // 2 + 1)
                if kk >= 12 and kk % 2 == 0:
                    s_reduce(kk // 2 - 6)
                if kk < KKT - 2:
                    build_w(2 * kk + 4)
                    build_w(2 * kk + 5)

            evict_a(0)

            # evict_b lags ~7 slabs mid-run (rides out the S collective
            # latency), catches up two-per-slab from slab 10.  The backlog is
            # drained at the TOP of each iteration so it overlaps that slab's
            # matmuls, leaving only slab 15's evict_b after the last matmul.
            # ypool ring safety: live ytile slabs stay <= 8 = 32/SUB, and the
            # freeing evict_b is always emitted before the reusing evict_a.
            done_b = 0
            for s in range(1, NS):
                for kk in slab_mms(s, [nc.sync, nc.scalar]):
                    pass
                want = s - 6 if s < 10 else 2 * (s - 10) + 5
                while done_b <= min(want, s - 1):
                    evict_b(done_b)
                    done_b += 1
                evict_a(s)
                if s <= 4:
                    for i in range(4 * s - 2, min(4 * s + 2, 16)):
                        s_reduce(i)
                    if s == 4:
                        s_finish()
            while done_b < NS:
                evict_b(done_b)
                done_b += 1

    nc.compile()
    return nc


def kernel(x, q_values, q_scales, l_values, l_scales, r_values, r_scales, bias,
           _trace=False):
    from concourse.bass_utils import run_bass_kernel_spmd

    bf16 = ml_dtypes.bfloat16
    e4m3 = ml_dtypes.float8_e4m3

    if "m" not in _module_cache:
        _module_cache["m"] = _build_module()
    nc = _module_cache["m"]

    # host-side marshaling (layout + dtype + power-of-two scaling only)
    x = np.asarray(x, dtype=np.float32)
    q_values = np.asarray(q_values)
    q_scales = np.asarray(q_scales, np.float32)
    l_values = np.asarray(l_values)
    l_scales = np.asarray(l_scales, np.float32)
    r_values = np.asarray(r_values)
    r_scales = np.asarray(r_scales, np.float32)
    bias = np.asarray(bias, np.float32)

    # x*16 as e4m3, tiled [NS, KKT, P, 2, MS]: plane ko holds i = kk*256+ko*128+p
    xs = np.clip(x * 16.0, -240.0, 240.0)
    xt8 = np.ascontiguousarray(
        xs.reshape(NS, MS, KKT, 2, P).transpose(0, 2, 4, 3, 1)
    ).astype(e4m3)
    # bf16 raw-row x for the S reduction, per-core shard below
    xb_all = x.astype(bf16)

    rs_full = np.repeat(r_scales, D_IN // r_scales.shape[1], axis=1)
    rv_f = r_values.astype(np.float32)
    # wbr[p, ch, j, 0, :] = r codes, [.., 1, :] = broadcast r scales
    wbr = np.empty((P, RCH, RG, 2, RCW), np.float32)
    for ch in range(RCH):
        cs = slice(ch * RCW, (ch + 1) * RCW)
        for j in range(RG):
            wbr[:, ch, j, 0, :] = rv_f[j * P:(j + 1) * P, cs]
            wbr[:, ch, j, 1, :] = rs_full[j * P:(j + 1) * P, cs]
    wbr = wbr.astype(e4m3)

    in_maps = []
    for c in range(NCORES):
        sl = slice(c * OC, (c + 1) * OC)
        qt_c = q_values[sl].T.astype(np.float32)            # [D_IN, OC]
        qst_c = (q_scales[sl].T / 16.0).astype(np.float32)  # [KT, OC]
        ltv_c = l_values[sl].T.astype(np.float32)           # [RANK, OC]
        lst_c = (l_scales[sl].T / 16.0).astype(np.float32)  # [RG, OC]

        lcod = np.empty((P, RG, OC), np.float32)
        for j in range(RG):
            lcod[:, j, :] = ltv_c[j * P:(j + 1) * P, :]
        wbq = np.empty((P, KT, 2, OC), np.float32)
        for k in range(KT):
            wbq[:, k, 0, :] = qt_c[k * P:(k + 1) * P, :]
            wbq[:, k, 1, :] = np.broadcast_to(qst_c[k], (P, OC))

        in_maps.append({
            "xt": xt8,
            "lcod": lcod.astype(e4m3),
            "lsc": np.ascontiguousarray(lst_c.reshape(1, RG, OC)).astype(bf16),
            "wbr": wbr,
            "wbq": wbq.astype(e4m3),
            "xb": np.ascontiguousarray(
                xb_all[c * SHTOK:(c + 1) * SHTOK].reshape(SHT, P, 2, D_IN // 2)),
            "biasr": np.ascontiguousarray(bias[sl].reshape(1, OC)).astype(bf16),
        })

    res = run_bass_kernel_spmd(
        nc, in_maps, core_ids=list(range(NCORES)), trace=_trace
    )
    global last_result
    last_result = res
    return np.concatenate([r["y"] for r in res.results], axis=1)
